# revision 11
# baseline (speedup 1.0000x reference)
"""GQA attention (dense_transformer) Trainium2 Bass kernel, 8 NeuronCores.

Sharding: hybrid tensor-parallel. Batch (B=2) split over two groups of 4
cores; within a group the 24 Q heads are split 6-per-core. Head pairs are
assigned [kvA-pair0, kvA-pair1, kvB-pair] so every core sees the same
(uniform) program: pair m uses duplicated-KV tile [0,0,1][m]. Each core
computes Q/K/V projections for its heads, RoPE, causal attention, and its
partial Wo product; a ReduceScatter over each 4-core group sums Wo
partials, leaving each core a distinct 512-row output slice.

The device program is identical on all 8 cores (SPMD); only input data
(weight slices, batch slice, rope tables) differs per core.
"""
import sys
if "/opt/trn_rl_repo" not in sys.path:
    sys.path.insert(0, "/opt/trn_rl_repo")

import numpy as np
import ml_dtypes

HID, NH, NKV, HD, BASE = 1536, 24, 6, 64, 10000.0
B, S = 2, 2048
N_CORES = 8
HPC = 6                 # q heads per core
NPAIR = HPC // 2        # head pairs per core (3)
LOCD = HPC * HD         # local head dims (384)
NK = HID // 128         # contraction chunks (12)
QC = S // 512           # q-chunks of 512 (4)
NKT = S // 128          # key tiles (16)
NKV_LOC = 2             # distinct kv heads per core
PAIR_KT = (0, 0, 1)     # which local kv tile each pair uses
PAIR_VOFF = (0, 0, 65)  # column offset into v_sb per pair
SCALE = 1.0 / np.sqrt(HD)   # folded into Wq on host (0.125, exact in bf16)

# per 4-core group: full kv head + (lone kv head, lone pair index)
CORE_KV = [(0, 2, 0), (1, 2, 1), (3, 5, 0), (4, 5, 1)]


def core_heads(g4):
    """Global Q-head order for core group-slot g4 (pairs: kvA p0, kvA p1, kvB lone)."""
    fkv, lkv, lp = CORE_KV[g4]
    return [4 * fkv, 4 * fkv + 1, 4 * fkv + 2, 4 * fkv + 3,
            4 * lkv + 2 * lp, 4 * lkv + 2 * lp + 1]


BF = ml_dtypes.bfloat16

_NC_CACHE = {}


def build_nc(with_rs=True):
    import concourse.tile as tile
    from concourse import bacc, mybir

    f32 = mybir.dt.float32
    bf16 = mybir.dt.bfloat16
    AF = mybir.ActivationFunctionType
    ALU = mybir.AluOpType

    nc = bacc.Bacc("TRN2", target_bir_lowering=False, debug=False,
                   num_devices=N_CORES)

    xT = nc.dram_tensor("xT", [HID, S], bf16, kind="ExternalInput")
    wqT = nc.dram_tensor("wqT", [HID, LOCD], bf16, kind="ExternalInput")
    wkTd = nc.dram_tensor("wkTd", [HID, NKV_LOC * 128], bf16, kind="ExternalInput")
    wvTa = nc.dram_tensor("wvTa", [HID, NKV_LOC * 65], bf16, kind="ExternalInput")
    woT = nc.dram_tensor("woT", [LOCD, HID], bf16, kind="ExternalInput")
    cosT = nc.dram_tensor("cosT", [128, S], bf16, kind="ExternalInput")
    sinT = nc.dram_tensor("sinT", [128, S], bf16, kind="ExternalInput")
    rotPT = nc.dram_tensor("rotPT", [128, 128], bf16, kind="ExternalInput")
    masks = nc.dram_tensor("masks", [4, 128, 512], bf16, kind="ExternalInput")
    out = nc.dram_tensor("out", [512, HID], f32, kind="ExternalOutput")
    wo_part = [nc.dram_tensor(f"wo_part{e}", [S, 512], f32) for e in range(3)]
    rs_outs = [nc.dram_tensor(f"rs_out{e}", [512, 512], f32) for e in range(3)]

    with tile.TileContext(nc) as tc:
        with (
            tc.tile_pool(name="const", bufs=1) as const,
            tc.tile_pool(name="persist", bufs=1) as persist,
            tc.tile_pool(name="work", bufs=3) as work,
            tc.tile_pool(name="probs_p", bufs=4) as probs_p,
            tc.tile_pool(name="div_p", bufs=2) as div_p,
            tc.tile_pool(name="ps_a", bufs=2, space="PSUM") as ps_a,
            tc.tile_pool(name="ps_sc", bufs=2, space="PSUM") as ps_sc,
            tc.tile_pool(name="ps_av", bufs=2, space="PSUM") as ps_av,
        ):
            # ---- constants ----
            cos_sb = const.tile([128, S], bf16, tag="cos", name="cos")
            sin_sb = const.tile([128, S], bf16, tag="sin", name="sin")
            rot_sb = const.tile([128, 128], bf16, tag="rot", name="rot")
            nc.sync.dma_start(cos_sb[:], cosT[:])
            nc.sync.dma_start(sin_sb[:], sinT[:])
            nc.sync.dma_start(rot_sb[:], rotPT[:])
            mask_sb = []
            for d in range(4):
                mt = const.tile([128, 512], bf16, tag=f"mask{d}", name=f"mask{d}")
                nc.sync.dma_start(mt[:], masks[d])
                mask_sb.append(mt)

            # ---- weight/activation loads (spread across DMA queues) ----
            xT_sb = [persist.tile([128, S], bf16, tag=f"x{k}", name=f"x{k}") for k in range(NK)]
            wq_sb = [persist.tile([128, LOCD], bf16, tag=f"wq{k}", name=f"wq{k}") for k in range(NK)]
            wk_sb = [persist.tile([128, NKV_LOC * 128], bf16, tag=f"wk{k}", name=f"wk{k}") for k in range(NK)]
            wv_sb = [persist.tile([128, NKV_LOC * 65], bf16, tag=f"wv{k}", name=f"wv{k}") for k in range(NK)]
            for k in range(NK):
                sl = slice(k * 128, (k + 1) * 128)
                nc.sync.dma_start(xT_sb[k][:], xT[sl, :])
                nc.sync.dma_start(wk_sb[k][:], wkTd[sl, :])
            for k in range(NK):
                sl = slice(k * 128, (k + 1) * 128)
                nc.sync.dma_start(wv_sb[k][:], wvTa[sl, :])
            for k in range(NK):
                sl = slice(k * 128, (k + 1) * 128)
                nc.sync.dma_start(wq_sb[k][:], wqT[sl, :])
            wo_sb = [persist.tile([128, HID], bf16, tag=f"wo{k}", name=f"wo{k}") for k in range(NPAIR)]
            for k in range(NPAIR):
                nc.sync.dma_start(wo_sb[k][:], woT[k * 128:(k + 1) * 128, :])

            kt_ro = [persist.tile([128, S], bf16, tag=f"ktro{m}", name=f"ktro{m}") for m in range(NKV_LOC)]
            qt_ro = [persist.tile([128, S], bf16, tag=f"qtro{m}", name=f"qtro{m}") for m in range(NPAIR)]
            v_sb = [persist.tile([128, NKV_LOC * 65], bf16, tag=f"v{r}", name=f"v{r}") for r in range(NKT)]
            at_sb = [persist.tile([128, S], bf16, tag=f"at{m}", name=f"at{m}") for m in range(NPAIR)]

            # ---- projections + rope (K first to unlock attention early) ----
            def proj_rope(w_sb_list, dest, m):
                for s4 in range(QC):
                    cols = slice(s4 * 512, (s4 + 1) * 512)
                    pp = ps_a.tile([128, 512], f32, tag="pp", name="pp")
                    for k in range(NK):
                        nc.tensor.matmul(
                            pp[:], w_sb_list[k][:, m * 128:(m + 1) * 128],
                            xT_sb[k][:, cols],
                            start=(k == 0), stop=(k == NK - 1))
                    raw = work.tile([128, 512], bf16, tag="raw", name="raw")
                    nc.vector.tensor_copy(raw[:], pp[:])
                    rp = ps_a.tile([128, 512], f32, tag="pp", name="pp")
                    nc.tensor.matmul(rp[:], rot_sb[:], raw[:], start=True, stop=True)
                    t1 = work.tile([128, 512], bf16, tag="t1", name="t1")
                    nc.vector.tensor_tensor(t1[:], rp[:], sin_sb[:, cols], op=ALU.mult)
                    dsl = dest[:, cols]
                    nc.vector.tensor_tensor(dsl, raw[:], cos_sb[:, cols], op=ALU.mult)
                    nc.vector.tensor_tensor(dsl, dsl, t1[:], op=ALU.add)

            for m in range(NKV_LOC):
                proj_rope(wk_sb, kt_ro[m], m)
            for r in range(NKT):
                vp = ps_a.tile([128, NKV_LOC * 65], f32, tag="pp", name="pp")
                for k in range(NK):
                    nc.tensor.matmul(
                        vp[:], xT_sb[k][:, r * 128:(r + 1) * 128], wv_sb[k][:],
                        start=(k == 0), stop=(k == NK - 1))
                nc.vector.tensor_copy(v_sb[r][:], vp[:])
                ones_cols = v_sb[r].rearrange("p (a c) -> p a c", c=65)[:, :, 64:65]
                nc.vector.memset(ones_cols, 1.0)
            for m in range(NPAIR):
                proj_rope(wq_sb, qt_ro[m], m)

            # ---- attention ----
            for m in range(NPAIR):
                ktm = kt_ro[PAIR_KT[m]]
                voff = PAIR_VOFF[m]
                for qc in range(QC):
                    nkt = 4 * qc + 4
                    qcols = slice(qc * 512, (qc + 1) * 512)
                    av = [ps_av.tile([65, 512], f32, tag="av", name="av") for _ in range(2)]
                    for ch in range(nkt // 2):
                        sc = []
                        for hh in range(2):
                            hsl = slice(hh * 64, (hh + 1) * 64)
                            st = ps_sc.tile([128, 1024], f32, tag="sc", name="sc")
                            for i2 in range(2):
                                kt = ch * 2 + i2
                                nc.tensor.matmul(
                                    st[:, i2 * 512:(i2 + 1) * 512],
                                    ktm[hsl, kt * 128:(kt + 1) * 128],
                                    qt_ro[m][hsl, qcols],
                                    start=True, stop=True)
                            sc.append(st)
                        for hh in range(2):
                            pr = probs_p.tile([128, 1024], bf16, tag="probs", name="probs")
                            nc.scalar.activation(pr[:], sc[hh][:], AF.Exp)
                            for i2 in range(2):
                                kt = ch * 2 + i2
                                d = kt - 4 * qc
                                psl = pr[:, i2 * 512:(i2 + 1) * 512]
                                if d >= 0:
                                    nc.vector.tensor_tensor(
                                        psl, psl, mask_sb[d][:], op=ALU.mult)
                            for i2 in range(2):
                                kt = ch * 2 + i2
                                nc.tensor.matmul(
                                    av[hh][:],
                                    v_sb[kt][:, voff:voff + 65],
                                    pr[:, i2 * 512:(i2 + 1) * 512],
                                    start=(kt == 0), stop=(kt == nkt - 1))
                    for hh in range(2):
                        # custom DVE/GPSIMD ops require base-partition-0 APs on
                        # HW: move the denominator row to partition 0 first
                        den0 = div_p.tile([1, 512], f32, tag="den0", name="den0")
                        nc.vector.tensor_copy(den0[:], av[hh][64:65, :])
                        rec = div_p.tile([1, 512], f32, tag="rec", name="rec")
                        nc.vector.reciprocal_approx_fast(rec[:], den0[:])
                        recb = div_p.tile([64, 512], f32, tag="recb", name="recb")
                        nc.gpsimd.partition_broadcast(recb[:], rec[:])
                        if hh == 0:
                            nc.vector.tensor_tensor(
                                at_sb[m][0:64, qcols], av[hh][0:64, :], recb[:],
                                op=ALU.mult)
                        else:
                            tmp = div_p.tile([64, 512], bf16, tag="tmp", name="tmp")
                            nc.vector.tensor_tensor(
                                tmp[:], av[hh][0:64, :], recb[:], op=ALU.mult)
                            nc.vector.tensor_copy(at_sb[m][64:128, qcols], tmp[:])

            # ---- Wo partial + chunked ReduceScatter ----
            for e in range(HID // 512):
                for qt in range(NKT):
                    wp = ps_a.tile([128, 512], f32, tag="pp", name="pp")
                    for kk in range(NPAIR):
                        nc.tensor.matmul(
                            wp[:], at_sb[kk][:, qt * 128:(qt + 1) * 128],
                            wo_sb[kk][:, e * 512:(e + 1) * 512],
                            start=(kk == 0), stop=(kk == NPAIR - 1))
                    ob = work.tile([128, 512], f32, tag="ob", name="ob")
                    nc.vector.tensor_copy(ob[:], wp[:])
                    eng = nc.sync
                    eng.dma_start(
                        wo_part[e][qt * 128:(qt + 1) * 128, :], ob[:])
                if with_rs:
                    nc.gpsimd.collective_compute(
                        "ReduceScatter", ALU.add,
                        replica_groups=[[0, 1, 2, 3], [4, 5, 6, 7]],
                        ins=[wo_part[e][:]], outs=[rs_outs[e][:]])
                    nc.sync.dma_start(out[:, e * 512:(e + 1) * 512], rs_outs[e][:])
                else:
                    nc.sync.dma_start(out[:, e * 512:(e + 1) * 512],
                                      wo_part[e][0:512, :])

    nc.compile()
    return nc


def host_inputs(hidden_states, position_ids, Wq, Wk, Wv, Wo):
    """Build the 8 per-core input maps."""
    hs = np.asarray(hidden_states, dtype=np.float32)
    pos = np.asarray(position_ids).astype(np.int64)
    Wq = np.asarray(Wq, dtype=np.float32)
    Wk = np.asarray(Wk, dtype=np.float32)
    Wv = np.asarray(Wv, dtype=np.float32)
    Wo = np.asarray(Wo, dtype=np.float32)

    inv = 1.0 / (BASE ** (np.arange(0, HD, 2, dtype=np.float32) / HD))  # [32]
    cosT_b, sinT_b = [], []
    for b in range(B):
        emb = pos[b][:, None].astype(np.float32) * inv[None, :]  # [S, 32]
        emb = np.concatenate([emb, emb], axis=1)                 # [S, 64]
        ct = np.cos(emb).T
        st = np.sin(emb).T
        cosT_b.append(np.vstack([ct, ct]).astype(BF))
        sinT_b.append(np.vstack([st, st]).astype(BF))

    # rotate-half matrix (lhsT layout): rot = P @ x, rotPT[d, dd] = P[dd, d]
    R = np.zeros((64, 64), np.float32)
    for dd in range(32):
        R[dd, dd + 32] = -1.0
        R[dd + 32, dd] = 1.0
    P128 = np.zeros((128, 128), np.float32)
    P128[:64, :64] = R
    P128[64:, 64:] = R
    rotPT = P128.T.astype(BF)

    kk = np.arange(128)[:, None]
    qq = np.arange(512)[None, :]
    masks = np.stack([((kk + 128 * d) <= qq).astype(np.float32) for d in range(4)])
    masks = masks.astype(BF)

    Wq_s = (Wq * SCALE).astype(np.float32)
    in_maps = []
    for c in range(N_CORES):
        b, g4 = c // 4, c % 4
        heads = core_heads(g4)
        fkv, lkv, _ = CORE_KV[g4]
        # Q weight rows in local head order
        wq_loc = np.vstack([Wq_s[h * HD:(h + 1) * HD, :] for h in heads])
        wqT_h = np.ascontiguousarray(wq_loc.T).astype(BF)
        # duplicated KV tiles: local kv 0 = full kv, local kv 1 = lone kv
        kblocks, vblocks = [], []
        for kv in (fkv, lkv):
            wk_kv = Wk[kv * HD:(kv + 1) * HD, :]
            kblocks += [wk_kv, wk_kv]
            wv_kv = Wv[kv * HD:(kv + 1) * HD, :]
            vblocks += [wv_kv, np.zeros((1, HID), np.float32)]
        wkTd_h = np.ascontiguousarray(np.vstack(kblocks).T).astype(BF)  # [HID, 256]
        wvTa_h = np.ascontiguousarray(np.vstack(vblocks).T).astype(BF)  # [HID, 130]
        wo_loc = np.hstack([Wo[:, h * HD:(h + 1) * HD] for h in heads])
        woT_h = np.ascontiguousarray(wo_loc.T).astype(BF)               # [LOCD, HID]
        xT_h = np.ascontiguousarray(hs[b].T).astype(BF)
        in_maps.append({
            "xT": xT_h, "wqT": wqT_h, "wkTd": wkTd_h, "wvTa": wvTa_h,
            "woT": woT_h, "cosT": cosT_b[b], "sinT": sinT_b[b],
            "rotPT": rotPT, "masks": masks,
        })
    return in_maps


def assemble(results):
    out_full = np.empty((B, S, HID), dtype=np.float32)
    for c in range(N_CORES):
        b, g4 = c // 4, c % 4
        out_full[b, g4 * 512:(g4 + 1) * 512, :] = results[c]["out"]
    return out_full


def kernel(hidden_states, position_ids, Wq, Wk, Wv, Wo):
    from concourse.bass_utils import run_bass_kernel_spmd
    if "nc" not in _NC_CACHE:
        _NC_CACHE["nc"] = build_nc(with_rs=True)
    nc = _NC_CACHE["nc"]
    in_maps = host_inputs(hidden_states, position_ids, Wq, Wk, Wv, Wo)
    res = run_bass_kernel_spmd(nc, in_maps, core_ids=list(range(N_CORES)))
    return assemble(res.results)
